# revision 22
# baseline (speedup 1.0000x reference)
"""2-layer GCN + dense layers + mean-pool on 8 trn2 NeuronCores (Bass/Tile).

Math: GCNConv out = D^-1/2 (A+I) D^-1/2 (h W) + b factorizes into
  table = dinv * (h W)            (dense, per node)
  agg[d] = sum_{e: dst=d} table[src_e]   (pure 0/1 scatter-add, self loops
                                          are ordinary edges)
  out[d] = dinv[d] * agg[d] + b
so no per-edge norm work is needed.

Sharding: dst-range shard (12500 nodes/core). Each core gathers table rows
for its own edges from a LOCAL full copy of the table.
  Launch 1 (per core): builds the full conv1 table locally (replicated
    feature-major dense chain: cheaper than a cross-core collective),
    aggregates conv1 for its shard, post stage + dense chain -> outputs its
    shard of the conv2 table.
  Host: concatenates the 8 shards into the full conv2 table ("all-gather").
  Launch 2 (per core): aggregates conv2, final dense + graph-pool partials.
  Host: sums partials + final bias.

Aggregation kernel: edges sorted by (src-chunk, dst-window), padded per
(chunk, window) to 128-groups equal across cores (SPMD: one program).
dma_gather (int16 idxs, fp16 128-elem rows) pulls 128 edge rows per group
into SBUF partitions; a one-hot [128 edges x 128 dst-slots] lhsT built on
DVE (iota is_equal dstslot) scatter-adds them into a PSUM window via the
tensor engine; windows evacuate into an SBUF f32 accumulator.

Table layout: row r(v) = (v//32767)*32768 + (v%32767); local row 32767 of
each 32768-row chunk is a reserved zero row so pad edges in any chunk can
gather zeros. Table dtype fp16 (f32 PSUM accumulation everywhere).
"""

import os
import sys

sys.path.insert(0, "/opt/trn_rl_repo")

import numpy as np

import concourse.bass as bass
import concourse.tile as tile
from concourse import bacc, mybir
from concourse.bass_utils import run_bass_kernel_spmd

F32 = mybir.dt.float32
F16 = mybir.dt.float16
I16 = mybir.dt.int16
AF = mybir.ActivationFunctionType
ALU = mybir.AluOpType

N = int(os.environ.get("KSIM_N", "100000"))
F = 128
NOUT = 64
NG = 64
NCORES = 8
NSH = N // NCORES
WIN = 128
WPC = (NSH + WIN - 1) // WIN
SHPAD = WPC * WIN
CRE = 32767            # real rows per chunk
NCHUNK = (N + CRE) // CRE
TAB = NCHUNK * 32768   # table rows
NTP = -(-N // 512) * 4          # node tiles (512-multiple of nodes)
NPAD = NTP * 128
GCALL = 96             # gather groups per dma_gather call

LAST_EXEC_NS = None
LAST_INFO = {}


def _rowmap(v):
    v = np.asarray(v, np.int64)
    return (v // CRE) * 32768 + (v % CRE)


# ----------------------------------------------------------------------------
# host-side graph prep
# ----------------------------------------------------------------------------
def _prep(src, dst, batch):
    src = np.asarray(src, np.int64)
    dst = np.asarray(dst, np.int64)
    batch = np.asarray(batch, np.int64)

    deg = np.bincount(dst, minlength=N).astype(np.float64) + 1.0
    dinv = (1.0 / np.sqrt(deg)).astype(np.float32)

    loops = np.arange(N, dtype=np.int64)
    s_all = np.concatenate([src, loops])
    d_all = np.concatenate([dst, loops])

    core = d_all // NSH
    chunk = s_all // CRE
    iloc = (s_all % CRE).astype(np.int16)
    dloc = d_all - core * NSH
    w_all = dloc // WIN
    slot = (dloc % WIN).astype(np.float32)

    order = np.lexsort((w_all, chunk, core))
    key = (core * NCHUNK + chunk) * WPC + w_all
    key_s = key[order]
    iloc_s = iloc[order]
    slot_s = slot[order]

    nk = NCORES * NCHUNK * WPC
    cnt = np.bincount(key, minlength=nk).reshape(NCORES, NCHUNK, WPC)
    G = np.ceil(cnt.max(axis=0) / 128.0).astype(np.int64)  # [NCHUNK, WPC]

    sched = []
    for ch in range(NCHUNK):
        sched.append([(w, int(G[ch, w])) for w in range(WPC) if G[ch, w] > 0])
    Gc = [sum(g for _, g in s) for s in sched]
    GT = sum(Gc)
    EPAD = GT * 128

    bounds = np.searchsorted(key_s, np.arange(nk + 1))
    idx_streams = np.full((NCORES, EPAD), 32767, np.int16)  # pad -> zero row
    slot_streams = np.zeros((NCORES, EPAD), np.float32)
    for c in range(NCORES):
        o = 0
        for ch in range(NCHUNK):
            for w, g in sched[ch]:
                k = (c * NCHUNK + ch) * WPC + w
                b0, b1 = bounds[k], bounds[k + 1]
                n = b1 - b0
                idx_streams[c, o : o + n] = iloc_s[b0:b1]
                slot_streams[c, o : o + n] = slot_s[b0:b1]
                o += g * 128
        assert o == EPAD

    # SBUF layouts: edge i -> (partition i%16 col i//16) for idxs (replicated
    # x8 across partition groups), (partition i%128, col i//128) for slots
    idx2d = np.zeros((NCORES, 128, EPAD // 16), np.int16)
    dloc2d = np.zeros((NCORES, 128, GT), np.float32)
    for c in range(NCORES):
        a = idx_streams[c].reshape(-1, 16).T  # [16, EPAD/16]
        idx2d[c] = np.tile(a, (8, 1))
        dloc2d[c] = slot_streams[c].reshape(-1, 128).T  # [128, GT]

    counts = np.maximum(np.bincount(batch, minlength=NG), 1).astype(np.float64)

    # per-core post-scale dinv over shard (tile-major [128, WPC])
    dinv_sh = np.zeros((NCORES, 128, WPC), np.float32)
    g2d = np.zeros((NCORES, 128, WPC * NG), np.float16)
    for c in range(NCORES):
        ids = c * NSH + np.arange(SHPAD)
        ok = ids < (c + 1) * NSH
        v = np.where(ok, dinv[np.minimum(ids, N - 1)], 0.0)
        dinv_sh[c] = v.reshape(WPC, 128).T
        gmat = np.zeros((SHPAD, NG), np.float16)
        rid = ids[ok]
        gmat[ok, batch[rid]] = (1.0 / counts[batch[rid]]).astype(np.float16)
        g2d[c] = gmat.reshape(WPC, 128, NG).transpose(1, 0, 2).reshape(
            128, WPC * NG
        )

    # dense-stage dinv, tile-major over padded node tiles [128, NTP]
    dinv_pad = np.zeros(NPAD, np.float32)
    dinv_pad[:N] = dinv
    dinv2d = dinv_pad.reshape(NTP, 128).T.copy()

    return dict(
        dinv=dinv, sched=sched, Gc=Gc, GT=GT, idx2d=idx2d, dloc2d=dloc2d,
        counts=counts, dinv_sh=dinv_sh, g2d=g2d, dinv2d=dinv2d,
    )


# ----------------------------------------------------------------------------
# device program pieces
# ----------------------------------------------------------------------------
def _emit_agg(nc, tc, ctx, table_ap, idx_t, dloc_sb, iota_sb, acc, sched,
              wps):
    """Edge aggregation: gather + one-hot scatter-matmul into acc (SBUF f32
    [128, SHPAD]). table_ap: DRAM AP [TAB, F] fp16."""
    gath = ctx.enter_context(tc.tile_pool(name="gath", bufs=2))
    ohp = ctx.enter_context(tc.tile_pool(name="oh", bufs=8))
    idxp = ctx.enter_context(tc.tile_pool(name="idxp", bufs=3))

    nc.vector.memset(acc[:], 0.0)

    touched = set()
    gidx = 0  # global group counter
    eoff = 0  # global edge offset
    for ch in range(NCHUNK):
        tab_chunk = table_ap[ch]
        groups = []  # (w, j, glast) flattened for this chunk
        for w, g in sched[ch]:
            for j in range(g):
                groups.append((w, j, g))
        # issue per-call gathers
        ncal = (len(groups) + GCALL - 1) // GCALL
        gt_tiles = []
        for call in range(ncal):
            g0 = call * GCALL
            ng = min(GCALL, len(groups) - g0)
            it = idxp.tile([128, ng * 8], I16)
            a = (eoff + g0 * 128) // 16
            nc.sync.dma_start(it[:], idx_t[:, a : a + ng * 8])
            gt = gath.tile([128, GCALL * F], F16, tag="gath")
            nc.gpsimd.dma_gather(
                gt[:, : ng * F].rearrange("p (g e) -> p g e", e=F),
                tab_chunk,
                it[:],
                ng * 128,
                ng * 128,
                F,
                single_packet=False,
            )
            gt_tiles.append((gt, ng))

        wt = None
        for gi, (w, j, g) in enumerate(groups):
            gt, _ = gt_tiles[gi // GCALL]
            k = gi % GCALL
            oh = ohp.tile([128, 128], F16)
            nc.vector.tensor_scalar(
                oh[:], iota_sb[:], dloc_sb[:, gidx : gidx + 1], None,
                ALU.is_equal,
            )
            if j == 0:
                wt = wps.tile([128, F], F32)
            nc.tensor.matmul(
                wt[:], oh[:], gt[:, k * F : (k + 1) * F],
                start=(j == 0), stop=(j == g - 1),
            )
            if j == g - 1:
                dstsl = acc[:, w * F : (w + 1) * F]
                if w in touched:
                    nc.vector.tensor_add(dstsl, dstsl, wt[:])
                else:
                    nc.vector.tensor_copy(dstsl, wt[:])
                    touched.add(w)
            gidx += 1
        eoff += len(groups) * 128


def _emit_post(nc, tc, ctx, acc, dinv_sh_sb, ident_sb, bias_sb, hT, pst):
    """hT[:, w*128:(w+1)*128] = relu((acc_w * dinv_sh).T + bias)"""
    scp = ctx.enter_context(tc.tile_pool(name="scp", bufs=4))
    for w in range(WPC):
        sc = scp.tile([128, 128], F16)
        nc.vector.tensor_scalar(
            sc[:], acc[:, w * F : (w + 1) * F], dinv_sh_sb[:, w : w + 1],
            None, ALU.mult,
        )
        pt = pst.tile([128, 128], F16)
        nc.tensor.transpose(pt[:], sc[:], ident_sb[:])
        nc.scalar.activation(
            hT[:, w * F : (w + 1) * F], pt[:], AF.Relu, bias=bias_sb[:, 0:1]
        )


def _table_write_rows(t):
    """DMA row ranges for dense tile t (128 nodes at t*128), applying the
    chunk row remap; returns list of (table_row0, src_row0, nrows)."""
    v0 = t * 128
    out = []
    done = 0
    while done < 128:
        v = v0 + done
        ch, lo = divmod(v, CRE)
        n = min(128 - done, CRE - lo)
        out.append((ch * 32768 + lo, done, n))
        done += n
    return out


def _emit_dense_table(nc, tc, ctx, xT_ap, w1_sb, wc1_sb, b1_sb, dinv2d_sb,
                      ident_sb, table_ap, ps5, pst):
    """table[r(v)] = dinv[v] * (relu(x W1 + b1) Wc1)[v]  for all nodes."""
    xin = ctx.enter_context(tc.tile_pool(name="xin", bufs=3))
    hsb = ctx.enter_context(tc.tile_pool(name="hsb", bufs=3))
    tout = ctx.enter_context(tc.tile_pool(name="tout", bufs=4))

    # zero rows: local row 32767 of every chunk
    zt = tout.tile([1, F], F16, tag="zrow")
    nc.vector.memset(zt[:], 0.0)
    for ch in range(NCHUNK):
        nc.sync.dma_start(table_ap[ch][32767:32768, :], zt[:])
    # also zero the written-range tail rows beyond node N in chunk 3
    # (pad nodes have dinv 0 so their rows compute to 0 anyway)

    nd1 = int(os.environ.get("KD1N", str(NTP // 4)))
    for u in range(nd1):  # 196 chunks of 512 nodes
        xt = xin.tile([128, 512], F16)
        nc.sync.dma_start(xt[:], xT_ap[:, u * 512 : (u + 1) * 512])
        p1 = ps5.tile([128, 512], F32, tag="p")
        nc.tensor.matmul(p1[:], w1_sb[:], xt[:], start=True, stop=True)
        h1 = hsb.tile([128, 512], F16)
        nc.scalar.activation(h1[:], p1[:], AF.Relu, bias=b1_sb[:, 0:1])
        p2 = ps5.tile([128, 512], F32, tag="p")
        nc.tensor.matmul(p2[:], wc1_sb[:], h1[:], start=True, stop=True)
        g1 = hsb.tile([128, 512], F16)
        nc.scalar.activation(g1[:], p2[:], AF.Copy)
        for q in range(4):
            t = u * 4 + q
            pt = pst.tile([128, 128], F16)
            nc.tensor.transpose(pt[:], g1[:, q * 128 : (q + 1) * 128],
                                ident_sb[:])
            tt = tout.tile([128, F], F16, tag="trow")
            nc.scalar.activation(tt[:], pt[:], AF.Copy,
                                 scale=dinv2d_sb[:, t : t + 1])
            for r0, s0, nr in _table_write_rows(t):
                ch, lo = divmod(r0, 32768)
                nc.sync.dma_start(table_ap[ch][lo : lo + nr, :],
                                  tt[s0 : s0 + nr, :])


# ----------------------------------------------------------------------------
# builders
# ----------------------------------------------------------------------------
def _build_launch1(prep):
    import contextlib

    nc = bacc.Bacc("TRN2", target_bir_lowering=False, debug=False,
                   num_devices=NCORES)
    GT = prep["GT"]
    xT = nc.dram_tensor("xT", [128, NPAD], F16, kind="ExternalInput")
    w1 = nc.dram_tensor("w1", [128, 128], F16, kind="ExternalInput")
    wc1 = nc.dram_tensor("wc1", [128, 128], F16, kind="ExternalInput")
    wfc2 = nc.dram_tensor("wfc2", [128, 128], F16, kind="ExternalInput")
    wc2 = nc.dram_tensor("wc2", [128, 128], F16, kind="ExternalInput")
    b1 = nc.dram_tensor("b1", [128, 1], F32, kind="ExternalInput")
    bc1 = nc.dram_tensor("bc1", [128, 1], F32, kind="ExternalInput")
    bfc2 = nc.dram_tensor("bfc2", [128, 1], F32, kind="ExternalInput")
    dinv2d = nc.dram_tensor("dinv2d", [128, NTP], F32, kind="ExternalInput")
    dinv_sh = nc.dram_tensor("dinv_sh", [128, WPC], F32, kind="ExternalInput")
    idx2d = nc.dram_tensor("idx2d", [128, GT * 8], I16, kind="ExternalInput")
    dloc2d = nc.dram_tensor("dloc2d", [128, GT], F32, kind="ExternalInput")
    ident = nc.dram_tensor("ident", [128, 128], F16, kind="ExternalInput")
    iota = nc.dram_tensor("iota", [128, 128], F32, kind="ExternalInput")
    g2s = nc.dram_tensor("g2s", [SHPAD, 128], F16, kind="ExternalOutput")

    with tile.TileContext(nc) as tc, contextlib.ExitStack() as ctx:
        dram = ctx.enter_context(tc.tile_pool(name="dram", bufs=1,
                                              space="DRAM"))
        table = []
        for i in range(NCHUNK):
            tabt = dram.tile([32768, F], F16, tag="tab%d" % i)
            table.append(tabt)
        const = ctx.enter_context(tc.tile_pool(name="const", bufs=1))
        big = ctx.enter_context(tc.tile_pool(name="big", bufs=1))

        _ldn = [0]

        def ld(ap, shape, dtype):
            _ldn[0] += 1
            t = const.tile(shape, dtype, tag="c%d" % _ldn[0])
            nc.sync.dma_start(t[:], ap)
            return t

        w1_sb = ld(w1.ap(), [128, 128], F16)
        wc1_sb = ld(wc1.ap(), [128, 128], F16)
        wfc2_sb = ld(wfc2.ap(), [128, 128], F16)
        wc2_sb = ld(wc2.ap(), [128, 128], F16)
        b1_sb = ld(b1.ap(), [128, 1], F32)
        bc1_sb = ld(bc1.ap(), [128, 1], F32)
        bfc2_sb = ld(bfc2.ap(), [128, 1], F32)
        dinv2d_sb = ld(dinv2d.ap(), [128, NTP], F32)
        dinv_sh_sb = ld(dinv_sh.ap(), [128, WPC], F32)
        dloc_sb = ld(dloc2d.ap(), [128, GT], F32)

        ident_sb = ld(ident.ap(), [128, 128], F16)
        iota_sb = ld(iota.ap(), [128, 128], F32)
        acc = big.tile([128, SHPAD], F32)
        h2T = big.tile([128, SHPAD], F16)

        ps5 = ctx.enter_context(tc.tile_pool(name="ps5", bufs=2,
                                             space="PSUM"))
        pst = ctx.enter_context(tc.tile_pool(name="pst", bufs=2,
                                             space="PSUM"))
        wps = ctx.enter_context(tc.tile_pool(name="wps", bufs=3,
                                             space="PSUM"))

        stage = int(os.environ.get("KSTAGE", "4"))
        tabs = [t[:] for t in table]
        _emit_dense_table(nc, tc, ctx, xT.ap(), w1_sb, wc1_sb, b1_sb,
                          dinv2d_sb, ident_sb, tabs, ps5, pst)
        if stage >= 2:
            _emit_agg(nc, tc, ctx, tabs, idx2d.ap(), dloc_sb, iota_sb,
                      acc, prep["sched"], wps)
        else:
            nc.vector.memset(acc[:], 0.0)
        if stage >= 3:
            _emit_post(nc, tc, ctx, acc, dinv_sh_sb, ident_sb, bc1_sb, h2T,
                       pst)
        else:
            nc.vector.memset(h2T[:], 0.0)

        # D2: g2s = dinv_sh * (relu(h2 Wfc2 + bfc2) Wc2), feature-major
        hsb = ctx.enter_context(tc.tile_pool(name="hsbb", bufs=3))
        tout = ctx.enter_context(tc.tile_pool(name="toutb", bufs=4))
        for u in range(SHPAD // 512 + 1):  # 24x512 + 1x256
            c0 = u * 512
            cw = min(512, SHPAD - c0)
            if cw <= 0:
                break
            p1 = ps5.tile([128, 512], F32, tag="p")
            nc.tensor.matmul(p1[:, :cw], wfc2_sb[:], h2T[:, c0 : c0 + cw],
                             start=True, stop=True)
            h3 = hsb.tile([128, 512], F16, tag="h")
            nc.scalar.activation(h3[:, :cw], p1[:, :cw], AF.Relu,
                                 bias=bfc2_sb[:, 0:1])
            p2 = ps5.tile([128, 512], F32, tag="p")
            nc.tensor.matmul(p2[:, :cw], wc2_sb[:], h3[:, :cw], start=True,
                             stop=True)
            g2 = hsb.tile([128, 512], F16, tag="h")
            nc.scalar.activation(g2[:, :cw], p2[:, :cw], AF.Copy)
            for q in range(cw // 128):
                t = u * 4 + q
                pt = pst.tile([128, 128], F16)
                nc.tensor.transpose(pt[:], g2[:, q * 128 : (q + 1) * 128],
                                    ident_sb[:])
                tt = tout.tile([128, F], F16)
                nc.scalar.activation(tt[:], pt[:], AF.Copy,
                                     scale=dinv_sh_sb[:, t : t + 1])
                nc.sync.dma_start(g2s.ap()[t * 128 : (t + 1) * 128, :], tt[:])

    nc.compile()
    return nc


def _build_launch2(prep):
    import contextlib

    nc = bacc.Bacc("TRN2", target_bir_lowering=False, debug=False,
                   num_devices=NCORES)
    GT = prep["GT"]
    g2p = [nc.dram_tensor("g2p%d" % i, [32768, F], F16,
                          kind="ExternalInput") for i in range(NCHUNK)]
    wfc = nc.dram_tensor("wfc", [128, NOUT], F16, kind="ExternalInput")
    bc2 = nc.dram_tensor("bc2", [128, 1], F32, kind="ExternalInput")
    dinv_sh = nc.dram_tensor("dinv_sh", [128, WPC], F32, kind="ExternalInput")
    idx2d = nc.dram_tensor("idx2d", [128, GT * 8], I16, kind="ExternalInput")
    dloc2d = nc.dram_tensor("dloc2d", [128, GT], F32, kind="ExternalInput")
    g2d = nc.dram_tensor("g2d", [128, WPC * NG], F16, kind="ExternalInput")
    ident = nc.dram_tensor("ident", [128, 128], F16, kind="ExternalInput")
    iota = nc.dram_tensor("iota", [128, 128], F32, kind="ExternalInput")
    pool = nc.dram_tensor("pool", [NG, NOUT], F32, kind="ExternalOutput")

    with tile.TileContext(nc) as tc, contextlib.ExitStack() as ctx:
        const = ctx.enter_context(tc.tile_pool(name="const", bufs=1))
        big = ctx.enter_context(tc.tile_pool(name="big", bufs=1))

        _ldn = [0]

        def ld(ap, shape, dtype):
            _ldn[0] += 1
            t = const.tile(shape, dtype, tag="c%d" % _ldn[0])
            nc.sync.dma_start(t[:], ap)
            return t

        wfc_sb = ld(wfc.ap(), [128, NOUT], F16)
        bc2_sb = ld(bc2.ap(), [128, 1], F32)
        dinv_sh_sb = ld(dinv_sh.ap(), [128, WPC], F32)
        dloc_sb = ld(dloc2d.ap(), [128, GT], F32)
        g2d_sb = ld(g2d.ap(), [128, WPC * NG], F16)
        ident_sb = ld(ident.ap(), [128, 128], F16)
        iota_sb = ld(iota.ap(), [128, 128], F32)

        acc = big.tile([128, SHPAD], F32)
        h4T = big.tile([128, SHPAD], F16)

        pst = ctx.enter_context(tc.tile_pool(name="pst", bufs=2,
                                             space="PSUM"))
        wps = ctx.enter_context(tc.tile_pool(name="wps", bufs=3,
                                             space="PSUM"))

        _emit_agg(nc, tc, ctx, [g.ap() for g in g2p], idx2d.ap(), dloc_sb,
                  iota_sb, acc, prep["sched"], wps)
        _emit_post(nc, tc, ctx, acc, dinv_sh_sb, ident_sb, bc2_sb, h4T, pst)

        # D3 + pool
        psd = ctx.enter_context(tc.tile_pool(name="psd", bufs=2, space="PSUM"))
        osb = ctx.enter_context(tc.tile_pool(name="osb", bufs=4))
        psp = ctx.enter_context(tc.tile_pool(name="psp", bufs=1, space="PSUM"))
        poolps = psp.tile([NG, NOUT], F32)
        for w in range(WPC):
            pd = psd.tile([128, NOUT], F32)
            nc.tensor.matmul(pd[:], h4T[:, w * F : (w + 1) * F], wfc_sb[:],
                             start=True, stop=True)
            ot = osb.tile([128, NOUT], F16)
            nc.scalar.activation(ot[:], pd[:], AF.Copy)
            nc.tensor.matmul(poolps[:], g2d_sb[:, w * NG : (w + 1) * NG],
                             ot[:], start=(w == 0), stop=(w == WPC - 1),
                             skip_group_check=True)
        pres = osb.tile([NG, NOUT], F32, tag="pres")
        nc.vector.tensor_copy(pres[:], poolps[:])
        nc.sync.dma_start(pool.ap(), pres[:])

    nc.compile()
    return nc


# ----------------------------------------------------------------------------
# identity matrix without device iota tricks
# ----------------------------------------------------------------------------
def _np16(x):
    return np.ascontiguousarray(x, np.float16)


def kernel(x, src, dst, batch, W_fc1, b_fc1, W_c1, b_c1, W_fc2, b_fc2, W_c2,
           b_c2, W_fc, b_fc):
    global LAST_EXEC_NS, LAST_INFO
    x = np.asarray(x, np.float32)
    prep = _prep(src, dst, batch)
    trace = os.environ.get("KERNEL_TRACE", "0") == "1"

    xT = np.zeros((128, NPAD), np.float16)
    xT[:, :N] = x.T
    col = lambda b: np.ascontiguousarray(
        np.asarray(b, np.float32).reshape(128, 1))
    ident = np.eye(128, dtype=np.float16)
    iota = np.tile(np.arange(128, dtype=np.float32), (128, 1))

    nc1 = _build_launch1(prep)
    in_maps1 = []
    for c in range(NCORES):
        in_maps1.append({
            "xT": xT, "w1": _np16(W_fc1), "wc1": _np16(W_c1),
            "wfc2": _np16(W_fc2), "wc2": _np16(W_c2),
            "b1": col(b_fc1), "bc1": col(b_c1), "bfc2": col(b_fc2),
            "dinv2d": prep["dinv2d"], "dinv_sh": prep["dinv_sh"][c],
            "idx2d": prep["idx2d"][c], "dloc2d": prep["dloc2d"][c],
            "ident": ident, "iota": iota,
        })
    import time as _time
    r1 = run_bass_kernel_spmd(nc1, in_maps1, core_ids=list(range(NCORES)),
                              trace=trace)
    t1_ns = None
    if os.environ.get("KERNEL_TIME", "0") == "1":
        from concourse.timeline_sim import TimelineSim
        tl = TimelineSim(nc1, trace=False)
        tl.simulate()
        t1_ns = int(tl.time)

    # host "all-gather": assemble conv2 table with row remap
    g2p = np.zeros((TAB, F), np.float16)
    rows = _rowmap(np.arange(N))
    for c in range(NCORES):
        sh = r1.results[c]["g2s"][:NSH]
        g2p[rows[c * NSH : (c + 1) * NSH]] = sh

    nc2 = _build_launch2(prep)
    in_maps2 = []
    for c in range(NCORES):
        im2 = {"g2p%d" % i: g2p[i * 32768 : (i + 1) * 32768]
               for i in range(NCHUNK)}
        im2.update({
            "wfc": _np16(W_fc), "bc2": col(b_c2),
            "dinv_sh": prep["dinv_sh"][c], "idx2d": prep["idx2d"][c],
            "dloc2d": prep["dloc2d"][c], "g2d": prep["g2d"][c],
            "ident": ident, "iota": iota,
        })
        in_maps2.append(im2)
    r2 = run_bass_kernel_spmd(nc2, in_maps2, core_ids=list(range(NCORES)),
                              trace=trace)
    t2_ns = None
    if os.environ.get("KERNEL_TIME", "0") == "1":
        from concourse.timeline_sim import TimelineSim
        tl = TimelineSim(nc2, trace=False)
        tl.simulate()
        t2_ns = int(tl.time)

    out = np.zeros((NG, NOUT), np.float64)
    for c in range(NCORES):
        out += r2.results[c]["pool"].astype(np.float64)
    out = out + np.asarray(b_fc, np.float64)[None, :]

    t1 = r1.exec_time_ns or t1_ns
    t2 = r2.exec_time_ns or t2_ns
    LAST_EXEC_NS = (t1 or 0) + (t2 or 0)
    LAST_INFO = {"t1": t1, "t2": t2, "GT": prep["GT"]}
    return out.astype(np.float32)
